# revision 1
# baseline (speedup 1.0000x reference)
"""Trainium2 Bass kernel for nn_ContextLearner (gnn_message_passing).

Per-graph transformer block over 1024 graphs x 100 nodes:
  emb gather -> LN1 -> QKV -> masked softmax attention -> proj
  -> attn-mean weighted combine -> LN2 -> FC

Sharding: data-parallel, 128 graphs per core across 8 NeuronCores.
Embedding table + weights replicated per core. Gather happens on-device
via indirect DMA (SWDGE).

Numerics: bf16 matmuls with fp32 PSUM accumulation; LN stats / softmax
in fp32. LN gains/biases folded into adjacent matmul weights on host
(exact algebra). rsqrt computed as Exp(-0.5*Ln(x)) so the whole kernel
stays in the single ACT table set `natural_log_exp_and_others`.
"""

import numpy as np
import ml_dtypes

import concourse.bass as bass
import concourse.tile as tile
from concourse import mybir
from concourse.bass_utils import run_bass_kernel_spmd

F32 = mybir.dt.float32
BF16 = mybir.dt.bfloat16
I32 = mybir.dt.int32

N_CORES = 8
NT = 100          # tokens (neighbors) per graph
DIM = 512         # channel dim C
H = 4             # heads
HD = 128          # head dim
VOCAB = 100001    # emb rows
E = 256           # emb dim
SCALE = HD ** -0.5

AF = mybir.ActivationFunctionType
ALU = mybir.AluOpType
AX = mybir.AxisListType


def _bf16(a):
    return np.ascontiguousarray(a.astype(ml_dtypes.bfloat16))


def build_program(n_graphs, have_qb_qk, have_qb_v, have_pb, have_fb,
                  n_process=None, repeat=1):
    """Emit the per-core program. All cores run the identical program on
    different data. n_graphs must be a multiple of 8. n_process (for
    timing-calibration builds) limits how many graphs are computed."""
    assert n_graphs % 8 == 0
    if n_process is None:
        n_process = n_graphs
    n_chunks = n_process // 8
    n_tail = (n_graphs + 31) // 32  # tail chunks of 32 graphs (128 rows)
    n_tail_run = (n_process + 31) // 32

    nc = bass.Bass()

    emb = nc.declare_dram_parameter("emb", [VOCAB, E], BF16, False)
    idx = nc.declare_dram_parameter("idx", [NT, n_graphs, 2], I32, False)
    maskT = nc.declare_dram_parameter("maskT", [NT, n_graphs, NT], BF16, False)
    wqk = nc.declare_dram_parameter("wqk", [128, 4, 8, 128], BF16, False)
    wv = nc.declare_dram_parameter("wv", [128, 4, DIM], BF16, False)
    projT = nc.declare_dram_parameter("projT", [128, 4, DIM], BF16, False)
    fcT = nc.declare_dram_parameter("fcT", [128, 4, E], BF16, False)
    ident = nc.declare_dram_parameter("ident", [128, 128], BF16, False)
    if have_qb_qk:
        qb_qk = nc.declare_dram_parameter("qb_qk", [128, 8], F32, False)
    if have_qb_v:
        qb_v = nc.declare_dram_parameter("qb_v", [1, DIM], BF16, False)
    if have_pb:
        pb = nc.declare_dram_parameter("pb", [1, DIM], BF16, False)
    if have_fb:
        fb = nc.declare_dram_parameter("fb", [1, E], BF16, False)
    out = nc.declare_dram_parameter("out", [n_tail, 128, E], F32, True)

    with tile.TileContext(nc) as tc:
        const = tc.alloc_tile_pool(name="const", bufs=1)
        xp = tc.alloc_tile_pool(name="xp", bufs=3)
        mp = tc.alloc_tile_pool(name="mp", bufs=3)
        sp = tc.alloc_tile_pool(name="sp", bufs=4)
        wp = tc.alloc_tile_pool(name="wp", bufs=4)
        ps_qk = tc.alloc_tile_pool(name="ps_qk", bufs=2, space="PSUM")
        ps_at = tc.alloc_tile_pool(name="ps_at", bufs=3, space="PSUM")
        ps_big = tc.alloc_tile_pool(name="ps_big", bufs=2, space="PSUM")
        ps_xnt = tc.alloc_tile_pool(name="ps_xnt", bufs=1, space="PSUM")

        # ---- constants ----
        wqk_sb = const.tile([128, 4, 8, 128], BF16, tag="wqk")
        nc.sync.dma_start(out=wqk_sb[:], in_=wqk[:])
        wv_sb = const.tile([128, 4, DIM], BF16, tag="wv")
        nc.sync.dma_start(out=wv_sb[:], in_=wv[:])
        projT_sb = const.tile([128, 4, DIM], BF16, tag="projT")
        nc.sync.dma_start(out=projT_sb[:], in_=projT[:])
        fcT_sb = const.tile([128, 4, E], BF16, tag="fcT")
        nc.sync.dma_start(out=fcT_sb[:], in_=fcT[:])
        ident_sb = const.tile([128, 128], BF16, tag="ident")
        nc.sync.dma_start(out=ident_sb[:], in_=ident[:])
        idx_sb = const.tile([NT, n_graphs, 2], I32, tag="idx")
        nc.sync.dma_start(out=idx_sb[:], in_=idx[:])
        eps_sb = const.tile([128, 1], F32, tag="eps")
        nc.vector.memset(eps_sb[:], 1e-5)
        wc_all = const.tile([128, n_tail, DIM], F32, tag="wc_all")
        nc.vector.memset(wc_all[:], 0.0)
        if have_qb_qk:
            qbqk_sb = const.tile([128, 8], F32, tag="qbqk")
            nc.sync.dma_start(out=qbqk_sb[:], in_=qb_qk[:])
        if have_qb_v or have_pb or have_fb:
            ones_sb = const.tile([1, 128], BF16, tag="ones")
            nc.vector.memset(ones_sb[:], 1.0)
        if have_qb_v:
            qbv_sb = const.tile([1, DIM], BF16, tag="qbv")
            nc.sync.dma_start(out=qbv_sb[:], in_=qb_v[:])
        if have_pb:
            pb_sb = const.tile([1, DIM], BF16, tag="pb")
            nc.sync.dma_start(out=pb_sb[:], in_=pb[:])
        if have_fb:
            fb_sb = const.tile([1, E], BF16, tag="fb")
            nc.sync.dma_start(out=fb_sb[:], in_=fb[:])

        for chunk in range(n_chunks * repeat):
            chunk = chunk % n_chunks
            g0 = chunk * 8
            # gather x for 8 graphs: x[t, g, e, :] = emb[idx[t, g0+g, e]]
            x_sb = xp.tile([NT, 8, 2, E], BF16, tag="x")
            for j in range(16):
                gj, ej = j // 2, j % 2
                nc.gpsimd.indirect_dma_start(
                    out=x_sb[:, gj, ej, :],
                    out_offset=None,
                    in_=emb[:, :],
                    in_offset=bass.IndirectOffsetOnAxis(
                        ap=idx_sb[:, g0 + gj, ej:ej + 1], axis=0),
                )
            mask_sb = mp.tile([NT, 8, NT], BF16, tag="mask")
            nc.sync.dma_start(out=mask_sb[:], in_=maskT[:, g0:g0 + 8, :])
            wcg_sb = mp.tile([H, 8, DIM], F32, tag="wcg")

            for pr in range(4):
                # ---------- LN1 for the pair ----------
                stats = sp.tile([NT, 2, 2, 6], F32, tag="stats")
                mv = sp.tile([NT, 2, 2], F32, tag="mv")
                for gl in range(2):
                    xg = x_sb[:, 2 * pr + gl, :, :]
                    for sub in range(2):
                        nc.vector.bn_stats(out=stats[:, gl, sub, :],
                                           in_=xg[:, sub, :])
                    nc.vector.bn_aggr(out=mv[:, gl, :], in_=stats[:, gl, :, :])
                lnt = sp.tile([NT, 2], F32, tag="lnt")
                nc.scalar.activation(out=lnt[:], in_=mv[:, :, 1],
                                     func=AF.Ln, bias=eps_sb[:NT], scale=1.0)
                rs = sp.tile([NT, 2], F32, tag="rs")
                nc.scalar.activation(out=rs[:], in_=lnt[:],
                                     func=AF.Exp, scale=-0.5)
                xn = sp.tile([NT, 2, DIM], BF16, tag="xn")
                for gl in range(2):
                    nc.vector.tensor_scalar(
                        out=xn[:, gl, :], in0=x_sb[:, 2 * pr + gl, :, :],
                        scalar1=mv[:, gl, 0:1], scalar2=rs[:, gl:gl + 1],
                        op0=ALU.subtract, op1=ALU.mult)

                # ---------- transpose xn -> xnT [c, kc, g, t] ----------
                xnT_sb = wp.tile([128, 4, 2, NT], BF16, tag="xnT")
                for gl in range(2):
                    xnT_ps = ps_xnt.tile([128, 4, NT], BF16, tag="xnt",
                                         space="PSUM")
                    for kc in range(4):
                        nc.tensor.transpose(
                            out=xnT_ps[:, kc, :],
                            in_=xn[:, gl, 128 * kc:128 * (kc + 1)],
                            identity=ident_sb[:NT, :NT])
                    nc.vector.tensor_copy(out=xnT_sb[:, :, gl, :],
                                          in_=xnT_ps[:])

                # ---------- q,k projections (pair-batched) ----------
                qk_sb = wp.tile([128, 8, 2 * NT], BF16, tag="qk")
                for half in range(2):
                    qkps = [ps_qk.tile([128, 2, 2 * NT], F32, tag="qkps",
                                       space="PSUM", name=f"qkps{_j}")
                            for _j in range(2)]
                    for j in range(4):
                        oc = 4 * half + j
                        for kc in range(4):
                            nc.tensor.matmul(
                                out=qkps[j // 2][:, j % 2, :],
                                lhsT=wqk_sb[:, kc, oc, :],
                                rhs=xnT_sb[:, kc, :, :],
                                start=(kc == 0), stop=(kc == 3))
                    for j in range(2):
                        o2 = 4 * half + 2 * j
                        if have_qb_qk:
                            for jj in range(2):
                                eng = nc.vector if j == 0 else nc.scalar
                                eng.tensor_scalar(
                                    out=qk_sb[:, o2 + jj, :],
                                    in0=qkps[j][:, jj, :],
                                    scalar1=qbqk_sb[:, o2 + jj:o2 + jj + 1],
                                    scalar2=None, op0=ALU.add)
                        elif j == 0:
                            nc.vector.tensor_copy(
                                out=qk_sb[:, o2:o2 + 2, :], in_=qkps[j][:])
                        else:
                            nc.scalar.activation(
                                out=qk_sb[:, o2:o2 + 2, :], in_=qkps[j][:],
                                func=AF.Copy)

                for gl in range(2):
                    g_loc = 2 * pr + gl
                    tok = slice(NT * gl, NT * (gl + 1))

                    # ---------- attention logits + softmax ----------
                    lg_ps = ps_at.tile([NT, H, NT], F32, tag="at",
                                       space="PSUM")
                    for h in range(H):
                        nc.tensor.matmul(
                            out=lg_ps[:, h, :],
                            lhsT=qk_sb[:, h, tok],
                            rhs=qk_sb[:, 4 + h, tok],
                            start=True, stop=True)
                    e_sb = sp.tile([NT, H, NT], BF16, tag="e")
                    nc.scalar.activation(out=e_sb[:], in_=lg_ps[:],
                                         func=AF.Exp, scale=SCALE)
                    em = sp.tile([NT, H, NT], BF16, tag="em")
                    dsum = sp.tile([NT, H], F32, tag="dsum")
                    for h in range(H):
                        nc.vector.tensor_tensor(
                            out=em[:, h, :], in0=e_sb[:, h, :],
                            in1=mask_sb[:, g_loc, :], op=ALU.mult)
                    nc.vector.tensor_reduce(out=dsum[:], in_=em[:],
                                            axis=AX.X, op=ALU.add)
                    rd = sp.tile([NT, H], BF16, tag="rd")
                    with nc.allow_low_precision(reason="softmax denom bf16"):
                        nc.vector.reciprocal(out=rd[:], in_=dsum[:])
                    attn = sp.tile([NT, H, NT], BF16, tag="attn")
                    nc.vector.tensor_tensor(
                        out=attn[:], in0=em[:],
                        in1=rd[:].to_broadcast([NT, H, NT]),
                        op=ALU.mult)

                    # ---------- transpose attn ----------
                    at_ps = ps_at.tile([NT, H, NT], BF16, tag="at",
                                       space="PSUM")
                    for h in range(H):
                        nc.tensor.transpose(
                            out=at_ps[:, h, :], in_=attn[:, h, :],
                            identity=ident_sb[:NT, :NT])
                    attnT = sp.tile([NT, H, NT], BF16, tag="attnT")
                    nc.scalar.activation(out=attnT[:], in_=at_ps[:],
                                         func=AF.Copy)
                    ams = sp.tile([NT, H], F32, tag="ams")
                    nc.vector.tensor_reduce(out=ams[:], in_=attnT[:],
                                            axis=AX.X, op=ALU.add)
                    amT = sp.tile([NT, H], BF16, tag="amT")
                    nc.vector.tensor_scalar_mul(out=amT[:], in0=ams[:],
                                               scalar1=1.0 / NT)

                    # ---------- v projection ----------
                    v_ps = ps_big.tile([NT, DIM], F32, tag="big",
                                       space="PSUM")
                    for kc in range(4):
                        nc.tensor.matmul(
                            out=v_ps[:], lhsT=xnT_sb[:, kc, gl, :],
                            rhs=wv_sb[:, kc, :],
                            start=(kc == 0), stop=(kc == 3 and not have_qb_v))
                    if have_qb_v:
                        nc.tensor.matmul(out=v_ps[:], lhsT=ones_sb[:1, :NT],
                                         rhs=qbv_sb[:], start=False, stop=True)
                    v_sb = sp.tile([NT, DIM], BF16, tag="v")
                    nc.scalar.activation(out=v_sb[:], in_=v_ps[:],
                                         func=AF.Copy)

                    # ---------- y^T = v^T @ attn^T ----------
                    yT_ps = ps_at.tile([128, H, NT], F32, tag="at",
                                       space="PSUM")
                    for h in range(H):
                        nc.tensor.matmul(
                            out=yT_ps[:, h, :],
                            lhsT=v_sb[:, HD * h:HD * (h + 1)],
                            rhs=attnT[:, h, :],
                            start=True, stop=True)
                    yT_sb = sp.tile([128, H, NT], BF16, tag="yT")
                    nc.vector.tensor_copy(out=yT_sb[:], in_=yT_ps[:])

                    # ---------- proj ----------
                    yo_ps = ps_big.tile([NT, DIM], F32, tag="big",
                                        space="PSUM")
                    for kc in range(4):
                        nc.tensor.matmul(
                            out=yo_ps[:], lhsT=yT_sb[:, kc, :],
                            rhs=projT_sb[:, kc, :],
                            start=(kc == 0), stop=(kc == 3 and not have_pb))
                    if have_pb:
                        nc.tensor.matmul(out=yo_ps[:], lhsT=ones_sb[:1, :NT],
                                         rhs=pb_sb[:], start=False, stop=True)
                    yo_sb = sp.tile([NT, DIM], BF16, tag="yo")
                    nc.scalar.activation(out=yo_sb[:], in_=yo_ps[:],
                                         func=AF.Copy)

                    # ---------- wc = amT.T @ yo ----------
                    wc_ps = ps_big.tile([H, DIM], F32, tag="big", space="PSUM")
                    nc.tensor.matmul(out=wc_ps[:], lhsT=amT[:], rhs=yo_sb[:],
                                     start=True, stop=True)
                    nc.scalar.activation(
                        out=wcg_sb[:, 2 * pr + gl, :], in_=wc_ps[:],
                        func=AF.Copy)

            t, jj = chunk // 4, chunk % 4
            nc.sync.dma_start(out=wc_all[32 * jj:32 * jj + 32, t, :],
                              in_=wcg_sb[:])

        # ---------- tail: LN2 + FC, batched 128 rows per tail chunk ----------
        for t in range(n_tail_run):
            tst = sp.tile([128, 2, 6], F32, tag="tstats")
            tmv = sp.tile([128, 2], F32, tag="tmv")
            for sub in range(2):
                nc.vector.bn_stats(out=tst[:, sub, :],
                                   in_=wc_all[:, t, 256 * sub:256 * (sub + 1)])
            nc.vector.bn_aggr(out=tmv[:], in_=tst[:])
            tln = sp.tile([128, 1], F32, tag="tln")
            nc.scalar.activation(out=tln[:], in_=tmv[:, 1:2], func=AF.Ln,
                                 bias=eps_sb[:], scale=1.0)
            trs = sp.tile([128, 1], F32, tag="trs")
            nc.scalar.activation(out=trs[:], in_=tln[:], func=AF.Exp,
                                 scale=-0.5)
            wcn = sp.tile([128, DIM], BF16, tag="wcn")
            nc.vector.tensor_scalar(out=wcn[:], in0=wc_all[:, t, :],
                                    scalar1=tmv[:, 0:1], scalar2=trs[:],
                                    op0=ALU.subtract, op1=ALU.mult)
            wcnT_ps = ps_at.tile([128, 4, 128], BF16, tag="at", space="PSUM")
            for kc in range(4):
                nc.tensor.transpose(out=wcnT_ps[:, kc, :],
                                    in_=wcn[:, 128 * kc:128 * (kc + 1)],
                                    identity=ident_sb[:])
            wcnT = sp.tile([128, 4, 128], BF16, tag="wcnT")
            nc.vector.tensor_copy(out=wcnT[:], in_=wcnT_ps[:])
            o_ps = ps_big.tile([128, E], F32, tag="big", space="PSUM")
            for kc in range(4):
                nc.tensor.matmul(out=o_ps[:], lhsT=wcnT[:, kc, :],
                                 rhs=fcT_sb[:, kc, :],
                                 start=(kc == 0), stop=(kc == 3 and not have_fb))
            if have_fb:
                nc.tensor.matmul(out=o_ps[:], lhsT=ones_sb[:1, :],
                                 rhs=fb_sb[:], start=False, stop=True)
            o_sb = sp.tile([128, E], F32, tag="osb")
            nc.scalar.activation(out=o_sb[:], in_=o_ps[:], func=AF.Copy)
            nc.sync.dma_start(out=out[t], in_=o_sb[:])

        for _pool in (ps_xnt, ps_big, ps_at, ps_qk, wp, sp, mp, xp, const):
            _pool.release()

    _split_matmul_waits(nc)
    return nc


_SPLIT_TYPES = (
    "InstMatmult", "InstLdweights", "InstTensorTensor", "InstTensorScalarPtr",
    "InstActivation", "InstTensorReduce", "InstTensorCopy", "InstBNStats",
    "InstBNStatsAggregate", "InstReciprocal", "InstTensorTensorReduce",
    "InstMemset", "InstDMACopy", "InstCopyPredicated", "InstSelect",
    "InstDrain",
)


def _split_matmul_waits(nc):
    """HW ISA slots hold a single sync-wait; move extras onto no-ops."""
    for fn in nc.m.functions:
        for blk in fn.blocks:
            new = []
            for inst in blk.instructions:
                si = getattr(inst, "sync_info", None)
                if (type(inst).__name__ in _SPLIT_TYPES
                        and si is not None and len(si.on_wait) > 1):
                    for w in si.on_wait[:-1]:
                        new.append(mybir.InstNoOp(
                            name=nc.get_next_instruction_name(),
                            engine=inst.engine,
                            bass_nofuse=True,
                            sync_info=mybir.SyncInfo(on_wait=[w],
                                                     on_update=[]),
                        ))
                    inst.sync_info = mybir.SyncInfo(
                        on_wait=[si.on_wait[-1]], on_update=si.on_update)
                new.append(inst)
            blk.instructions = new


def prep_host(inputs, n_graphs_total=1024, n_cores=N_CORES):
    """Fold LN params into weights, reshape/transpose inputs per core."""
    cons = np.asarray(inputs["connections"]).reshape(-1, NT, 2).astype(np.int32)
    mask = np.asarray(inputs["mask"]).reshape(-1, NT, NT).astype(np.float32)
    emb = np.asarray(inputs["emb"], dtype=np.float32)
    qkv_w = np.asarray(inputs["qkv_w"], dtype=np.float32)
    qkv_b = np.asarray(inputs["qkv_b"], dtype=np.float32)
    proj_w = np.asarray(inputs["proj_w"], dtype=np.float32)
    proj_b = np.asarray(inputs["proj_b"], dtype=np.float32)
    ln1_g = np.asarray(inputs["ln1_g"], dtype=np.float32)
    ln1_b = np.asarray(inputs["ln1_b"], dtype=np.float32)
    ln2_g = np.asarray(inputs["ln2_g"], dtype=np.float32)
    ln2_b = np.asarray(inputs["ln2_b"], dtype=np.float32)
    fc_w = np.asarray(inputs["fc_w"], dtype=np.float32)
    fc_b = np.asarray(inputs["fc_b"], dtype=np.float32)

    # fold LN1 gain/bias into qkv, LN2 gain/bias into fc (exact algebra)
    W = qkv_w * ln1_g[None, :]
    qb = qkv_b + qkv_w @ ln1_b
    Wf = fc_w * ln2_g[None, :]
    fb = fc_b + fc_w @ ln2_b

    # lhsT tiles for q,k: [c_in_chunk(128), kc, oc, m]
    wqk = _bf16(W[:1024].reshape(8, 128, 4, 128).transpose(3, 2, 0, 1))
    # rhs tiles for v: [c_in_chunk, kc, feat]
    wv = _bf16(W[1024:1536].T.reshape(4, 128, DIM).transpose(1, 0, 2))
    projT = _bf16(proj_w.T.reshape(4, 128, DIM).transpose(1, 0, 2))
    fcT = _bf16(Wf.T.reshape(4, 128, E).transpose(1, 0, 2))
    ident = _bf16(np.eye(128, dtype=np.float32))
    emb_bf = _bf16(emb)

    have_qb_qk = bool(np.any(qb[:1024]))
    have_qb_v = bool(np.any(qb[1024:]))
    have_pb = bool(np.any(proj_b))
    have_fb = bool(np.any(fb))

    shared = {"wqk": wqk, "wv": wv, "projT": projT, "fcT": fcT,
              "ident": ident, "emb": emb_bf}
    if have_qb_qk:
        shared["qb_qk"] = np.ascontiguousarray(
            qb[:1024].reshape(8, 128).T.astype(np.float32))
    if have_qb_v:
        shared["qb_v"] = _bf16(qb[1024:].reshape(1, DIM))
    if have_pb:
        shared["pb"] = _bf16(proj_b.reshape(1, DIM))
    if have_fb:
        shared["fb"] = _bf16(fb.reshape(1, E))

    gpc = n_graphs_total // n_cores
    in_maps = []
    for c in range(n_cores):
        sl = slice(c * gpc, (c + 1) * gpc)
        m = dict(shared)
        m["idx"] = np.ascontiguousarray(cons[sl].transpose(1, 0, 2))
        m["maskT"] = _bf16(mask[sl].transpose(1, 0, 2))
        in_maps.append(m)
    flags = (have_qb_qk, have_qb_v, have_pb, have_fb)
    return in_maps, flags, gpc


_CACHE = {}


def kernel(**inputs):
    n_total = np.asarray(inputs["connections"]).reshape(-1, NT, 2).shape[0]
    in_maps, flags, gpc = prep_host(inputs, n_total)
    key = (gpc,) + flags
    if key not in _CACHE:
        _CACHE[key] = build_program(gpc, *flags)
    nc = _CACHE[key]
    res = run_bass_kernel_spmd(nc, in_maps, list(range(N_CORES)))
    outs = []
    for r in res.results:
        o = r["out"]  # [n_tail, 128, 256]; row p = 32*jj + 8*h + g8
        nt = o.shape[0]
        o = o.reshape(nt, 4, 4, 8, E).transpose(0, 1, 3, 2, 4)
        outs.append(o.reshape(-1, 4, E))
    return np.concatenate(outs, axis=0).astype(np.float32)

